# revision 9
# baseline (speedup 1.0000x reference)
"""Trainium2 Bass kernel for the leveled-DAG GRU message-passing network.

Strategy (8-core SPMD):
  - Host computes graded level labels from E (L[dst] == L[src]+1 for every
    edge — holds for the generator's leveled DAG), shards each level's nodes
    round-robin by in-degree across the 8 cores, and builds slot-major gather
    index tables for the per-destination segment sums.
  - Hidden state is replicated across cores in fp16 (with per-level
    power-of-2 scaling to stay in fp16 range) via one AllGather per
    (layer, level).  hidden[src] rows are fetched with
    dma_gather(transpose=True), which lands messages directly in
    [H-on-partition, edge-slot] layout.
  - Segment sums are slot-major DVE adds; GRU matmuls run in fp16 on the
    tensor engine with fp32 PSUM accumulation ([gates, nodes] layout, so
    biases are per-partition ACT activation operands and sigmoid/tanh run on
    the scalar engine).
  - Each core computes 1/8 of every level; per-core outputs are returned
    transposed in fp16 and the host assembles/rescales the final fp32 array.
"""

import math

import numpy as np

import concourse.bacc as bacc
import concourse.mybir as mybir
import concourse.tile as tile
from concourse.bass_utils import run_bass_kernel_spmd

NCORES = 8
H = 256
PART = 128
F16 = mybir.dt.float16
F32 = mybir.dt.float32
I16 = mybir.dt.int16
AF = mybir.ActivationFunctionType


# ---------------------------------------------------------------------------
# host-side graph preprocessing
# ---------------------------------------------------------------------------


def _grade_labels(src, dst, n_nodes):
    L = np.zeros(n_nodes, dtype=np.int64)
    for _ in range(64):
        Lup = L.copy()
        np.maximum.at(Lup, dst, L[src] + 1)
        np.maximum.at(Lup, src, L[dst] - 1)
        if (Lup == L).all():
            break
        L = Lup
    else:
        raise AssertionError("grade_labels did not converge")
    assert (L[dst] == L[src] + 1).all(), "input graph is not graded"
    assert L.min() >= 0
    return L


class _Plan:
    pass


def _build_plan(E, n_nodes):
    src = np.asarray(E[0]).astype(np.int64)
    dst = np.asarray(E[1]).astype(np.int64)
    p = _Plan()
    p.src = src
    p.dst = dst

    L = _grade_labels(src, dst, n_nodes)
    T = int(L.max()) + 1

    indeg = np.zeros(n_nodes, dtype=np.int64)
    np.add.at(indeg, dst, 1)

    own = [[None] * NCORES for _ in range(T)]
    Ps = [0] * T
    for t in range(T):
        nodes_t = np.where(L == t)[0]
        order = np.argsort(-indeg[nodes_t], kind="stable")
        nodes_t = nodes_t[order]
        for c in range(NCORES):
            own[t][c] = nodes_t[c::NCORES]
        max_m = max(len(own[t][c]) for c in range(NCORES))
        Ps[t] = ((max_m + 1 + PART - 1) // PART) * PART
    P = max(Ps)
    assert all(x == P for x in Ps), f"non-uniform chunk sizes {Ps}"

    node_core = np.zeros(n_nodes, dtype=np.int64)
    node_pos = np.zeros(n_nodes, dtype=np.int64)
    for t in range(T):
        for c in range(NCORES):
            node_core[own[t][c]] = c
            node_pos[own[t][c]] = np.arange(len(own[t][c]))

    # edges grouped by dst
    order = np.argsort(dst, kind="stable")
    e_src = src[order]
    e_dst = dst[order]
    first = np.searchsorted(e_dst, np.arange(n_nodes))

    ntiles = P // PART
    slot_D = [None] * T
    slot_prefix = [None] * T
    num_idxs = [0] * T
    for t in range(1, T):
        D = np.zeros(ntiles, dtype=np.int64)
        for c in range(NCORES):
            deg = np.zeros(P, dtype=np.int64)
            m = len(own[t][c])
            deg[:m] = indeg[own[t][c]]
            D = np.maximum(D, deg.reshape(ntiles, PART).max(axis=1))
        slot_D[t] = D
        slot_prefix[t] = [int((D > s).sum()) for s in range(int(D.max(initial=0)))]
        num_idxs[t] = int(PART * D.sum())

    # block list per sweep: (offset, nblocks*128) per slot; split into two
    # gather calls of roughly equal size for pipelining
    blocks = [None] * T
    for t in range(1, T):
        off = 0
        lst = []
        for pref in slot_prefix[t]:
            nb = pref * PART
            lst.append((off, nb))
            off += nb
        blocks[t] = lst

    NIH = 0  # max per-call idx count (half a sweep)
    splits = [None] * T
    for t in range(1, T):
        ni = num_idxs[t]
        half = ni // 2
        cut = 0
        acc = 0
        for i, (off, nb) in enumerate(blocks[t]):
            if acc >= half:
                cut = i
                break
            acc += nb
            cut = i + 1
        c0 = sum(nb for _, nb in blocks[t][:cut])
        splits[t] = (cut, c0)
        NIH = max(NIH, c0, ni - c0)
    NIH = max(((NIH + PART - 1) // PART) * PART, PART)

    zero_rel = P - 1
    idx_tabs = []
    for t in range(1, T):
        per_core = []
        D = slot_D[t]
        for c in range(NCORES):
            nodes = own[t][c]
            m = len(nodes)
            deg = np.zeros(P, dtype=np.int64)
            st = np.zeros(P, dtype=np.int64)
            deg[:m] = indeg[nodes]
            st[:m] = first[nodes]
            blks = []
            for s in range(int(D.max(initial=0))):
                for g in range(ntiles):
                    if D[g] <= s:
                        continue
                    pos = np.arange(g * PART, (g + 1) * PART)
                    valid = deg[pos] > s
                    rel = np.full(PART, zero_rel, dtype=np.int64)
                    vpos = pos[valid]
                    if vpos.size:
                        srcs = e_src[st[vpos] + s]
                        rel[valid] = node_core[srcs] * P + node_pos[srcs]
                    blks.append(rel)
            idx = np.concatenate(blks) if blks else np.zeros(0, dtype=np.int64)
            assert idx.shape[0] == num_idxs[t]
            assert idx.max(initial=0) < 32768
            per_core.append(idx.astype(np.int16))
        idx_tabs.append(per_core)

    p.T = T
    p.P = P
    p.RT = T * P
    p.ntiles = ntiles
    p.own = own
    p.slot_prefix = slot_prefix
    p.blocks = blocks
    p.splits = splits
    p.num_idxs = num_idxs
    p.NIH = NIH
    p.idx_tabs = idx_tabs
    p.indeg = indeg
    return p


def _calibrate_scales(p, V, dense_w, dense_b, gru_w_ih, gru_w_hh, gru_b_ih,
                      gru_b_hh):
    """Exact fp32 per-level max |h| over both layers -> power-of-2 scales."""
    n_nodes = V.shape[0]
    T = p.T
    x = V.astype(np.float32) @ dense_w.T.astype(np.float32) + dense_b
    lvl_max = np.zeros(T)
    n_layers = gru_w_ih.shape[0]
    src, dst = p.src, p.dst
    # per-level edge masks (sorted by dst level)
    lvl_of = np.zeros(n_nodes, dtype=np.int64)
    for t in range(T):
        for c in range(NCORES):
            lvl_of[p.own[t][c]] = t
    e_order = np.argsort(lvl_of[dst], kind="stable")
    es, ed = src[e_order], dst[e_order]
    e_bounds = np.searchsorted(lvl_of[ed], np.arange(T + 1))
    h_all = np.zeros((n_nodes, H), dtype=np.float32)
    for layer in range(n_layers):
        w_ih = gru_w_ih[layer].astype(np.float32)
        w_hh = gru_w_hh[layer].astype(np.float32)
        b_ih = gru_b_ih[layer].astype(np.float32)
        b_hh = gru_b_hh[layer].astype(np.float32)
        h_all[:] = 0.0
        for t in range(T):
            nodes = np.where(lvl_of == t)[0]
            if nodes.size == 0:
                continue
            agg = np.zeros((n_nodes, H), dtype=np.float32)
            lo, hi = e_bounds[t], e_bounds[t + 1]
            if hi > lo:
                np.add.at(agg, ed[lo:hi], h_all[es[lo:hi]])
            a = agg[nodes]
            xi = x[nodes]
            gi = xi @ w_ih.T
            gh = a @ w_hh.T
            r = 1.0 / (1.0 + np.exp(-(gi[:, :H] + gh[:, :H] + b_ih[:H] + b_hh[:H])))
            z = 1.0 / (1.0 + np.exp(
                -(gi[:, H:2 * H] + gh[:, H:2 * H] + b_ih[H:2 * H] + b_hh[H:2 * H])))
            n = np.tanh(gi[:, 2 * H:] + b_ih[2 * H:]
                        + r * (gh[:, 2 * H:] + b_hh[2 * H:]))
            h = n + z * (a - n)
            h_all[nodes] = h
            lvl_max[t] = max(lvl_max[t], np.abs(h).max(initial=0.0))
        x = h_all.copy()
    scales = []
    for t in range(T):
        m = max(lvl_max[t], 1e-30)
        s = 2.0 ** max(0, int(math.ceil(math.log2(m / 32.0))))
        scales.append(float(s))
    return scales


# ---------------------------------------------------------------------------
# device program
# ---------------------------------------------------------------------------


def _build_program(plan, scales):
    T, P, RT, NIH = plan.T, plan.P, plan.RT, plan.NIH
    NT = plan.ntiles
    NImax = max(plan.num_idxs) if T > 1 else PART
    WI = ((NImax + 15) // 16)
    WI = ((WI + 7) // 8) * 8  # pad for alignment
    NTAB = max(T - 1, 1)
    REG = NCORES * P
    n_regions = T + (T - 1)

    nc = bacc.Bacc("TRN2", num_devices=NCORES)

    vt_in = nc.dram_tensor("vt_in", [PART, 2, RT], F16, kind="ExternalInput")
    idx_in = nc.dram_tensor("idx_in", [NTAB, PART, WI], I16, kind="ExternalInput")
    dwt_in = nc.dram_tensor("dwt_in", [PART, 2, H], F16, kind="ExternalInput")
    wih_in = nc.dram_tensor("wih_in", [2, PART, 2, 3 * H], F16,
                            kind="ExternalInput")
    whh_in = nc.dram_tensor("whh_in", [2, PART, 2, 3 * H], F16,
                            kind="ExternalInput")
    dbias_in = nc.dram_tensor("dbias_in", [PART, T, 2], F32, kind="ExternalInput")
    gbias_in = nc.dram_tensor("gbias_in", [PART, 2, 8], F32, kind="ExternalInput")
    nbias_in = nc.dram_tensor("nbias_in", [PART, 2, T, 4], F32,
                              kind="ExternalInput")

    out16 = nc.dram_tensor("out16", [PART, 2, RT], F16, kind="ExternalOutput")

    hidden = nc.dram_tensor("hidden", [n_regions * REG, H], F16, kind="Internal")
    agin = [
        nc.dram_tensor(f"agin{l}", [P, H], F16, kind="Internal") for l in range(2)
    ]

    def reg_base(layer, t):
        return (t if layer == 0 else T + t) * REG

    with tile.TileContext(nc) as tc:
        with (
            tc.tile_pool(name="consts", bufs=1) as cpool,
            tc.tile_pool(name="msgs", bufs=2) as mpool,
            tc.tile_pool(name="xv", bufs=2) as xpool,
            tc.tile_pool(name="work", bufs=2) as wpool,
            tc.tile_pool(name="hp0", bufs=3) as hp0pool,
            tc.tile_pool(name="hp1", bufs=2) as hp1pool,
            tc.tile_pool(name="rows", bufs=2) as rpool,
            tc.tile_pool(name="psum", bufs=4, space="PSUM") as ppool,
        ):
            # ---- constants ----
            dwt = cpool.tile([PART, 2, H], F16, tag="dwt")
            nc.sync.dma_start(dwt[:], dwt_in[:, :, :])
            wih = [cpool.tile([PART, 2, 3 * H], F16, tag=f"wih{l}", name=f"wih{l}")
                   for l in range(2)]
            whh = [cpool.tile([PART, 2, 3 * H], F16, tag=f"whh{l}", name=f"whh{l}")
                   for l in range(2)]
            for l in range(2):
                nc.sync.dma_start(wih[l][:], wih_in[l, :, :, :])
                nc.sync.dma_start(whh[l][:], whh_in[l, :, :, :])
            dbias = cpool.tile([PART, T, 2], F32, tag="dbias")
            nc.sync.dma_start(dbias[:], dbias_in[:, :, :])
            gbias = cpool.tile([PART, 2, 8], F32, tag="gbias")
            nc.sync.dma_start(gbias[:], gbias_in[:, :, :])
            nbias = cpool.tile([PART, 2, T, 4], F32, tag="nbias")
            nc.sync.dma_start(nbias[:], nbias_in[:, :, :, :])
            idxs = cpool.tile([PART, NTAB, WI], I16, tag="idxs")
            nc.sync.dma_start(
                idxs[:], idx_in.ap().rearrange("n p w -> p n w")
            )

            ncw = [512, P - 512]
            assert 0 < ncw[1] <= 512

            def compute_sweep(layer, t, hprev_x):
                S = scales[t]
                Sx = 1.0 if t == 0 else scales[t - 1]
                ratio = S / Sx
                wihl, whhl = wih[layer], whh[layer]

                # ---- gather (two pipelined halves) ----
                halves = []
                if t > 0:
                    ni = plan.num_idxs[t]
                    cut, c0 = plan.splits[t]
                    base = reg_base(layer, t - 1)
                    for (o0, cnt) in ((0, c0), (c0, ni - c0)):
                        if cnt == 0:
                            halves.append(None)
                            continue
                        mh = mpool.tile(
                            [PART, 2, cnt], F16, tag="msgs", name="msgs",
                        )
                        nc.gpsimd.dma_gather(
                            mh[:, :, :cnt],
                            hidden[base:base + REG, :],
                            idxs[:, t - 1, o0 // 16:(o0 + cnt) // 16],
                            cnt,
                            cnt,
                            H,
                            transpose=True,
                            single_packet=False,
                        )
                        halves.append((mh, o0, cnt))

                # ---- x tile (fp16, scaled by 1/Sx) ----
                if layer == 0:
                    vt = xpool.tile([PART, 2, P], F16, tag="vt")
                    nc.sync.dma_start(vt[:], vt_in[:, :, t * P:(t + 1) * P])
                    pxs = []
                    for m2 in range(2):
                        px = ppool.tile([PART, 2, 512], F32, tag="ps")
                        for ncid in range(2):
                            w = ncw[ncid]
                            c0_ = ncid * 512
                            for k in range(2):
                                nc.tensor.matmul(
                                    px[:, ncid, :w],
                                    dwt[:, k, m2 * 128:(m2 + 1) * 128],
                                    vt[:, k, c0_:c0_ + w],
                                    start=(k == 0),
                                    stop=(k == 1),
                                )
                        pxs.append(px)
                    # evacuate back into vt (it is dead after the matmuls)
                    xs = vt
                    for m2 in range(2):
                        for ncid in range(2):
                            w = ncw[ncid]
                            c0_ = ncid * 512
                            nc.scalar.activation(
                                xs[:, m2, c0_:c0_ + w],
                                pxs[m2][:, ncid, :w],
                                AF.Identity,
                                bias=dbias[:, t, m2:m2 + 1],
                                scale=1.0 / Sx,
                            )
                else:
                    xs = hprev_x
                    if ratio != 1.0:
                        nc.vector.tensor_scalar_mul(xs[:], xs[:], ratio)

                # ---- segment sum (fp16, scaled 1/Sx) ----
                aggs = wpool.tile([PART, 2, P], F16, tag="aggs")
                nc.vector.memset(aggs[:], 0.0)
                if t > 0:
                    cut, c0 = plan.splits[t]
                    for hi, (mh_info, blk_range) in enumerate(
                        zip(halves, (plan.blocks[t][:cut], plan.blocks[t][cut:]))
                    ):
                        if mh_info is None:
                            continue
                        mh, o0, cnt = mh_info
                        for (off, nb) in blk_range:
                            nc.vector.tensor_add(
                                aggs[:, :, :nb],
                                aggs[:, :, :nb],
                                mh[:, :, off - o0:off - o0 + nb],
                            )

                gb = gbias

                # ---- r/z gates ----
                r = wpool.tile([PART, 2, P], F16, tag="r")
                z = wpool.tile([PART, 2, P], F16, tag="z")
                for m in range(4):
                    prz = ppool.tile([PART, 2, 512], F32, tag="ps")
                    for ncid in range(2):
                        w = ncw[ncid]
                        c0_ = ncid * 512
                        for k in range(2):
                            nc.tensor.matmul(
                                prz[:, ncid, :w],
                                wihl[:, k, m * 128:(m + 1) * 128],
                                xs[:, k, c0_:c0_ + w],
                                start=(k == 0),
                                stop=False,
                            )
                        for k in range(2):
                            nc.tensor.matmul(
                                prz[:, ncid, :w],
                                whhl[:, k, m * 128:(m + 1) * 128],
                                aggs[:, k, c0_:c0_ + w],
                                start=False,
                                stop=(k == 1),
                            )
                    gate = r if m < 2 else z
                    kk = m % 2
                    for ncid in range(2):
                        w = ncw[ncid]
                        c0_ = ncid * 512
                        nc.scalar.activation(
                            gate[:, kk, c0_:c0_ + w],
                            prz[:, ncid, :w],
                            AF.Sigmoid,
                            bias=gb[:, layer, m:m + 1],
                            scale=Sx,
                        )

                # ---- n gate ----
                hn = wpool.tile([PART, 2, P], F16, tag="hn")
                gin = wpool.tile([PART, 2, P], F16, tag="gin")
                for kk in range(2):
                    m = 4 + kk
                    pgi = ppool.tile([PART, 2, 512], F32, tag="ps")
                    pgh = ppool.tile([PART, 2, 512], F32, tag="ps")
                    for ncid in range(2):
                        w = ncw[ncid]
                        c0_ = ncid * 512
                        for k in range(2):
                            nc.tensor.matmul(
                                pgi[:, ncid, :w],
                                wihl[:, k, m * 128:(m + 1) * 128],
                                xs[:, k, c0_:c0_ + w],
                                start=(k == 0),
                                stop=(k == 1),
                            )
                        for k in range(2):
                            nc.tensor.matmul(
                                pgh[:, ncid, :w],
                                whhl[:, k, m * 128:(m + 1) * 128],
                                aggs[:, k, c0_:c0_ + w],
                                start=(k == 0),
                                stop=(k == 1),
                            )
                    for ncid in range(2):
                        w = ncw[ncid]
                        c0_ = ncid * 512
                        nc.scalar.activation(
                            hn[:, kk, c0_:c0_ + w],
                            pgh[:, ncid, :w],
                            AF.Copy,
                        )
                        nc.scalar.activation(
                            gin[:, kk, c0_:c0_ + w],
                            pgi[:, ncid, :w],
                            AF.Copy,
                        )
                # v = gi' + r*gh'  (scaled domain)
                nc.vector.tensor_mul(hn[:], r[:], hn[:])
                nc.vector.tensor_add(gin[:], gin[:], hn[:])
                # r <- (b_ihn + r*b_hhn)/Sx   (host-prescaled per-sweep biases)
                for kk in range(2):
                    nc.vector.tensor_scalar(
                        r[:, kk, :],
                        r[:, kk, :],
                        nbias[:, layer, t, kk:kk + 1],
                        nbias[:, layer, t, 2 + kk:3 + kk],
                        mybir.AluOpType.mult,
                        mybir.AluOpType.add,
                    )
                nc.vector.tensor_add(gin[:], gin[:], r[:])
                n_t = wpool.tile([PART, 2, P], F16, tag="hn")  # reuse slot tag
                nc.scalar.activation(n_t[:], gin[:], AF.Tanh, scale=Sx)

                # ---- h' = f + z*(agg/ratio - f);  f = n/S ----
                hpool = hp0pool if layer == 0 else hp1pool
                hp = hpool.tile([PART, 2, P], F16, tag=f"hp{layer}")
                if ratio != 1.0:
                    nc.vector.tensor_scalar_mul(aggs[:], aggs[:], 1.0 / ratio)
                f_t = wpool.tile([PART, 2, P], F16, tag="gin")  # reuse slot tag
                nc.vector.tensor_scalar_mul(f_t[:], n_t[:], 1.0 / S)
                nc.vector.tensor_sub(aggs[:], aggs[:], f_t[:])
                nc.vector.tensor_mul(aggs[:], z[:], aggs[:])
                nc.vector.tensor_add(hp[:], f_t[:], aggs[:])
                nc.vector.memset(hp[:, :, P - 1:P], 0.0)

                # ---- store ----
                if layer == 0 or t < T - 1:
                    rows = rpool.tile([PART, NT, H], F16, tag="rows")
                    for j in range(2):
                        for g in range(NT):
                            nc.sync.dma_start(
                                rows[:, g, j * 128:(j + 1) * 128],
                                hp[:, j, g * 128:(g + 1) * 128],
                                transpose=True,
                            )
                    nc.sync.dma_start(
                        agin[layer].ap().rearrange("(g p) h -> p g h", p=PART),
                        rows[:],
                    )
                    base = reg_base(layer, t)
                    nc.gpsimd.collective_compute(
                        "AllGather",
                        mybir.AluOpType.bypass,
                        replica_groups=[list(range(NCORES))],
                        ins=[agin[layer][:]],
                        outs=[hidden[base:base + REG, :]],
                    )
                if layer == 1:
                    nc.sync.dma_start(out16[:, :, t * P:(t + 1) * P], hp[:])
                return hp

            import os as _os
            max_steps = int(_os.environ.get("KMAX_STEPS", str(T + 1)))
            hl0 = [None] * T
            skip_l1 = _os.environ.get("KSKIP_L1", "0") == "1"
            for s in range(min(T + 1, max_steps)):
                if s < T:
                    hl0[s] = compute_sweep(0, s, None)
                if s >= 1 and not skip_l1:
                    compute_sweep(1, s - 1, hl0[s - 1])
                    hl0[s - 1] = None

    nc.compile()
    return nc


# ---------------------------------------------------------------------------
# host-side input/output marshalling
# ---------------------------------------------------------------------------


def _pack_inputs(plan, scales, V, dense_w, dense_b, gru_w_ih, gru_w_hh,
                 gru_b_ih, gru_b_hh):
    T, P, RT = plan.T, plan.P, plan.RT
    NImax = max(plan.num_idxs) if T > 1 else PART
    WI = ((NImax + 15) // 16)
    WI = ((WI + 7) // 8) * 8
    NTAB = max(T - 1, 1)

    dwt = np.ascontiguousarray(
        dense_w.T.astype(np.float16).reshape(2, 128, H).transpose(1, 0, 2)
    )
    wih = np.stack([
        np.ascontiguousarray(
            gru_w_ih[l].T.astype(np.float16).reshape(2, 128, 3 * H)
            .transpose(1, 0, 2)
        )
        for l in range(2)
    ])
    whh = np.stack([
        np.ascontiguousarray(
            gru_w_hh[l].T.astype(np.float16).reshape(2, 128, 3 * H)
            .transpose(1, 0, 2)
        )
        for l in range(2)
    ])

    SSs = [1.0] + [scales[t] for t in range(T - 1)]
    dbias = np.zeros((128, T, 2), dtype=np.float32)
    for t in range(T):
        for m2 in range(2):
            dbias[:, t, m2] = dense_b[m2 * 128:(m2 + 1) * 128] / SSs[t]
    gbias = np.zeros((128, 2, 8), dtype=np.float32)
    nbias = np.zeros((128, 2, T, 4), dtype=np.float32)
    for l in range(2):
        bsum = gru_b_ih[l] + gru_b_hh[l]
        for m in range(4):
            gbias[:, l, m] = bsum[m * 128:(m + 1) * 128]
        for t in range(T):
            sx = SSs[t]
            for kk in range(2):
                nbias[:, l, t, kk] = (
                    gru_b_hh[l][2 * H + kk * 128:2 * H + (kk + 1) * 128] / sx
                )
                nbias[:, l, t, 2 + kk] = (
                    gru_b_ih[l][2 * H + kk * 128:2 * H + (kk + 1) * 128] / sx
                )

    in_maps = []
    for c in range(NCORES):
        vt = np.zeros((128, 2, RT), dtype=np.float16)
        for t in range(T):
            nodes = plan.own[t][c]
            m = len(nodes)
            if m:
                blk = V[nodes].astype(np.float16)
                blk = blk.reshape(m, 2, 128).transpose(2, 1, 0)
                vt[:, :, t * P:t * P + m] = blk
        idx = np.zeros((NTAB, 128, WI), dtype=np.int16)
        for t in range(1, T):
            tab = plan.idx_tabs[t - 1][c]
            ni = plan.num_idxs[t]
            if ni:
                wrapped = tab.reshape(ni // 16, 16).T
                idx[t - 1, :, :ni // 16] = np.tile(wrapped, (8, 1))
        in_maps.append({
            "vt_in": vt,
            "idx_in": idx,
            "dwt_in": dwt,
            "wih_in": wih,
            "whh_in": whh,
            "dbias_in": dbias,
            "gbias_in": gbias,
            "nbias_in": nbias,
        })
    return in_maps


def _unpack_output(plan, scales, results, n_nodes):
    T, P = plan.T, plan.P
    out = np.zeros((n_nodes, H), dtype=np.float32)
    for c in range(NCORES):
        o = results[c]["out16"]
        o = np.ascontiguousarray(o.transpose(2, 1, 0)).reshape(plan.RT, H)
        o = o.astype(np.float32)
        for t in range(T):
            nodes = plan.own[t][c]
            m = len(nodes)
            if m:
                out[nodes] = o[t * P:t * P + m] * scales[t]
    return out


_CACHE = {}


def kernel(V, E, dense_w, dense_b, gru_w_ih, gru_w_hh, gru_b_ih, gru_b_hh):
    V = np.asarray(V)
    E = np.asarray(E)
    dense_w = np.asarray(dense_w, dtype=np.float32)
    dense_b = np.asarray(dense_b, dtype=np.float32)
    gru_w_ih = np.asarray(gru_w_ih, dtype=np.float32)
    gru_w_hh = np.asarray(gru_w_hh, dtype=np.float32)
    gru_b_ih = np.asarray(gru_b_ih, dtype=np.float32)
    gru_b_hh = np.asarray(gru_b_hh, dtype=np.float32)
    n_nodes = V.shape[0]

    plan = _build_plan(E, n_nodes)
    scales = _calibrate_scales(
        plan, V, dense_w, dense_b, gru_w_ih, gru_w_hh, gru_b_ih, gru_b_hh
    )

    key = (plan.T, plan.P, plan.NIH, tuple(plan.num_idxs),
           tuple(tuple(sp) for sp in plan.slot_prefix if sp is not None),
           tuple(scales))
    if key not in _CACHE:
        _CACHE[key] = _build_program(plan, scales)
    nc = _CACHE[key]

    in_maps = _pack_inputs(plan, scales, V, dense_w, dense_b,
                           gru_w_ih, gru_w_hh, gru_b_ih, gru_b_hh)
    res = run_bass_kernel_spmd(nc, in_maps, core_ids=list(range(NCORES)))
    return _unpack_output(plan, scales, res.results, n_nodes)
